# revision 5
# baseline (speedup 1.0000x reference)
"""FP8 batch-matmul-dense kernel for Trainium2 (8 NeuronCores, batch-sharded).

Problem: out[b] = fp8qdq(x)[b] @ fp8qdq(w)[b] + bias[b]
  x: [32, 512, 2048] f32, w: [32, 2048, 2048] f32, bias: [32, 1, 2048] f32
  fp8qdq = torchao-style dynamic tensorwise scaling: s = 448/amax(|t|),
  q = e4m3fn(t*s), dq = q/s. Global (whole-tensor) amax.

Sharding: batch axis across 8 cores, 4 slices each (expert-parallel style).

Single fused NEFF, two logical phases:
  Phase A streams x (16MiB) then w (64MiB) at fp32, computing exact local
  amaxes on DVE. amax_x is AllReduce(max)'d across the 8 cores while w still
  streams; x is then PE-transposed and quantized into 4MiB of resident fp8
  lhsT codes and its 16MiB staging pool is released (stack-allocator reuse)
  for the phase-B pools. amax_w is AllReduce'd at the end of phase A.
  Phase B re-reads w (64MiB), quantizes (split ACT/DVE), runs DoubleRow fp8
  matmuls with fp32 PSUM accumulation, fused bias+rescale drains to bf16,
  and SWDGE output stores. The output is returned as bf16 and upcast on the
  host: the ~2^-9 relative rounding it adds is invisible at the 2e-2 gate
  and halves the output traffic + drain cost.

Collective/FIFO discipline (lessons from profiling):
  - The FIRST collective in a NEFF pays ~50us of setup; later ones ~15-25us.
    A dummy warmup AllReduce is fired at kernel start, hidden under the x
    loads, so the two real amax AllReduces ride the warm path.
  - Engine queues are strict FIFO: the sx scale ops are emitted ~20 reduces
    deep into the DVE stream so DVE never blocks on the collective wait.
  - Phase-B re-read DMAs depend only on stage-slot recycling (phase-A amax
    reduces), never on the collective result, so the ARw latency is hidden
    behind ~5MiB of prefetch plus quantization of already-landed tiles.

Quantization math (exact match to the reference): s' = 224/amax
  (= fl(448/amax)/2 exactly) because TRN fp8_e4m3 tops out at 240, not 448:
  the OCP e4m3fn lattice scaled by 1/2 lands exactly on the TRN lattice.
  Matmul runs on the raw fp8 codes (exact products, fp32 PSUM accum) and the
  output is rescaled by c = 1/(sx'*sw'). Scales are computed on-device with
  nc.vector.reciprocal; a 1-2 ulp deviation vs host fp32 divide perturbs
  ~1e-6 of the fp8 codes by 1 ulp - invisible at the 2e-2 gate.

Per-core HBM traffic: 16 (x) + 64 (w) + 64 (w re-read) + 8 (out bf16)
= 152MiB, one NEFF ramp, no exposed compute tail (x-transposes are prepaid
under the phase-A stream).
"""

import os
import sys

for _p in ("/root/.axon_site", "/root/.axon_site/_ro/trn_rl_repo", "/opt/trn_rl_repo"):
    if os.path.isdir(_p) and _p not in sys.path:
        sys.path.append(_p)

import numpy as np

import concourse.bass as bass
import concourse.bass_isa as bass_isa
import concourse.mybir as mybir
import concourse.tile as tile
from concourse import bacc
from concourse.bass_utils import run_bass_kernel_spmd
from concourse.masks import make_identity

# Problem shape (hardcoded per contest rules).
B, M, K, N = 32, 512, 2048, 2048
NCORES = 8
BL = B // NCORES          # 4 batch slices per core
P = 128
KT = K // P               # 16 k-tiles per batch
KP = KT // 2              # 8 k-pair tiles (256 rows) per batch
MT = M // P               # 4 m-tiles
NFREE = 512               # matmul moving free dim (one PSUM bank)
NT = N // NFREE           # 4 n-tiles
FP8_HALF_MAX = 224.0      # 448/2: OCP grid mapped onto TRN e4m3

F32 = mybir.dt.float32
BF16 = mybir.dt.bfloat16
FP8 = mybir.dt.float8e4

_cache = {}


def _build_fused_nc():
    nc = bacc.Bacc("TRN2", target_bir_lowering=False, debug=False, num_devices=NCORES)
    x = nc.dram_tensor("x", [BL, M, K], F32, kind="ExternalInput")
    w = nc.dram_tensor("w", [BL, K, N], F32, kind="ExternalInput")
    bias = nc.dram_tensor("bias", [BL, 1, N], F32, kind="ExternalInput")
    consts = nc.dram_tensor("consts", [1, 2], F32, kind="ExternalInput")
    out = nc.dram_tensor("out", [BL, M, N], BF16, kind="ExternalOutput")

    rg = [list(range(NCORES))]

    with tile.TileContext(nc) as tc:
        with (
            tc.tile_pool(name="small", bufs=1) as small,
            tc.tile_pool(name="acc", bufs=1) as accp,
            tc.tile_pool(name="xqt", bufs=1) as xqtp,
            tc.tile_pool(name="wstage", bufs=5) as wstage,
            tc.tile_pool(name="dram", bufs=6, space="DRAM") as dram,
            tc.tile_pool(name="trps", bufs=2, space="PSUM") as trps,
            tc.tile_pool(name="mmps", bufs=6, space="PSUM") as mmps,
        ):
            ident = small.tile([P, P], F32, name="ident")
            make_identity(nc, ident[:])
            cst = small.tile([1, 2], F32, name="cst")
            nc.sync.dma_start(cst[:], consts[0:1, :])
            # scl slots: 0=1/ax, 1=sx, 2=1/aw, 3=sw, 4=sx*sw, 5=c
            scl = small.tile([1, 8], F32, name="scl")
            axg = small.tile([1, 1], F32, name="axg")
            awg = small.tile([1, 1], F32, name="awg")
            cb = small.tile([P, 4], F32, name="cb")   # 0=sx, 1=sw, 2=c

            acc = accp.tile([P, 4 + BL * KT], F32, name="acc")
            red = accp.tile([P, 2], F32, name="red")
            par = accp.tile([P, 2], F32, name="par")

            # resident fp8 lhsT codes for all 4 batches: [k-part, kt, b*M+m]
            xqt = xqtp.tile([P, KT, BL * M], FP8, name="xqt")

            dum_in = dram.tile([1, 8], F32, name="dum_in")
            dum_out = dram.tile([1, 8], F32, name="dum_out")
            arx_in = dram.tile([1, 8], F32, name="arx_in")
            arx_out = dram.tile([1, 8], F32, name="arx_out")
            arw_in = dram.tile([1, 8], F32, name="arw_in")
            arw_out = dram.tile([1, 8], F32, name="arw_out")

            # warmup collective: pays the ~50us first-collective setup while
            # the x/w loads stream. Input is the consts tile (any data).
            nc.gpsimd.dma_start(dum_in[0:1, 0:2], cst[:])
            nc.gpsimd.collective_compute(
                "AllReduce", mybir.AluOpType.max, replica_groups=rg,
                ins=[dum_in.opt()], outs=[dum_out.opt()],
            )

            col = [4]

            def stage_load(b, kt, do_amax):
                """Load one k-tile row block w[b, kt*128:(kt+1)*128, :]."""
                ws = wstage.tile([P, N], F32, name="ws", tag="ws")
                nc.sync.dma_start(ws[:], w[b, kt * P:(kt + 1) * P, :])
                if do_amax:
                    nc.vector.tensor_reduce(
                        acc[:, col[0]:col[0] + 1], ws[:],
                        axis=mybir.AxisListType.XY, op=mybir.AluOpType.max,
                        apply_absolute_value=True,
                    )
                    col[0] += 1
                return ws

            staged_plan = [(b, kt) for b in range(BL) for kt in range(KT)]

            with tc.tile_pool(name="xbig", bufs=4) as xbig:
                # ---- x: load whole shard (4 x 4MiB), amax as tiles land ----
                xs_tiles = []
                for b in range(BL):
                    t = xbig.tile([P, 4, K], F32, name="xs", tag="xs")
                    src = x[b, :, :].rearrange("(p k) n -> k p n", p=4)
                    nc.sync.dma_start(t[:], src)
                    nc.vector.tensor_reduce(
                        acc[:, b:b + 1], t[:],
                        axis=mybir.AxisListType.XY, op=mybir.AluOpType.max,
                        apply_absolute_value=True,
                    )
                    xs_tiles.append(t)

                # ---- amax_x AllReduce trigger (result consumed later) ----
                nc.vector.tensor_reduce(
                    red[:, 0:1], acc[:, 0:BL],
                    axis=mybir.AxisListType.X, op=mybir.AluOpType.max,
                )
                nc.gpsimd.partition_all_reduce(
                    par[:, 0:1], red[:, 0:1], channels=P,
                    reduce_op=bass_isa.ReduceOp.max,
                )
                nc.gpsimd.dma_start(arx_in[0:1, 0:1], par[0:1, 0:1])
                nc.gpsimd.collective_compute(
                    "AllReduce", mybir.AluOpType.max, replica_groups=rg,
                    ins=[arx_in.opt()], outs=[arx_out.opt()],
                )
                nc.gpsimd.dma_start(axg[:], arx_out[0:1, 0:1])

                # first w loads pace the DVE queue past the collective wait
                for b_, kt_ in staged_plan[:20]:
                    stage_load(b_, kt_, do_amax=True)

                # sx = 224 / max(amax_x, 1e-12): by the time DVE reaches
                # these (20 reduces deep) the AllReduce result has landed.
                nc.vector.tensor_scalar_max(axg[:], axg[:], 1e-12)
                nc.vector.reciprocal(scl[0:1, 0:1], axg[:])
                nc.vector.tensor_scalar_mul(scl[0:1, 1:2], scl[0:1, 0:1], FP8_HALF_MAX)
                nc.gpsimd.partition_broadcast(cb[:, 0:1], scl[0:1, 1:2])
                sx_ap = cb[:, 0:1]

                for b_, kt_ in staged_plan[20:]:
                    stage_load(b_, kt_, do_amax=True)

                # ---- x: PE-transpose 128x128 blocks, quantize out of PSUM ----
                for b in range(BL):
                    for kt in range(KT):
                        ps = trps.tile([P, M], F32, name="tps", tag="tps")
                        for j in range(MT):
                            nc.tensor.transpose(
                                ps[:, j * P:(j + 1) * P],
                                xs_tiles[b][:, j, kt * P:(kt + 1) * P],
                                ident[:],
                            )
                        nc.scalar.activation(
                            xqt[:, kt, b * M:(b + 1) * M], ps[:],
                            mybir.ActivationFunctionType.Copy, scale=sx_ap,
                        )
            # xbig released: its 16MiB zone is reused by the pools below.

            with (
                tc.tile_pool(name="wq", bufs=16) as wqp,
                tc.tile_pool(name="ost", bufs=2) as ostp,
                tc.tile_pool(name="bias1", bufs=1) as bias1p,
                tc.tile_pool(name="biasb", bufs=1) as biasbp,
            ):
                # ---- amax_w AllReduce ----
                nc.vector.tensor_reduce(
                    red[:, 1:2], acc[:, BL:col[0]],
                    axis=mybir.AxisListType.X, op=mybir.AluOpType.max,
                )
                nc.gpsimd.partition_all_reduce(
                    par[:, 1:2], red[:, 1:2], channels=P,
                    reduce_op=bass_isa.ReduceOp.max,
                )
                nc.gpsimd.dma_start(arw_in[0:1, 0:1], par[0:1, 1:2])
                nc.gpsimd.collective_compute(
                    "AllReduce", mybir.AluOpType.max, replica_groups=rg,
                    ins=[arw_in.opt()], outs=[arw_out.opt()],
                )
                nc.gpsimd.dma_start(awg[:], arw_out[0:1, 0:1])
                # sw = 224 / max(amax_w, 1e-12); c = 1/(sx*sw)
                nc.vector.tensor_scalar_max(awg[:], awg[:], 1e-12)
                nc.vector.reciprocal(scl[0:1, 2:3], awg[:])
                nc.vector.tensor_scalar_mul(scl[0:1, 3:4], scl[0:1, 2:3], FP8_HALF_MAX)
                nc.vector.tensor_tensor(
                    scl[0:1, 4:5], scl[0:1, 1:2], scl[0:1, 3:4],
                    mybir.AluOpType.mult,
                )
                nc.vector.reciprocal(scl[0:1, 5:6], scl[0:1, 4:5])
                nc.gpsimd.partition_broadcast(cb[:, 1:2], scl[0:1, 3:4])
                nc.gpsimd.partition_broadcast(cb[:, 2:3], scl[0:1, 5:6])
                sw_ap = cb[:, 1:2]
                c_ap = cb[:, 2:3]

                # ---- phase B: re-read + quantize w, matmul, drain, store ----
                nq = 0
                for b in range(BL):
                    b1 = bias1p.tile([1, N], F32, name="b1", tag="b1")
                    nc.sync.dma_start(b1[:], bias[b, :, :])
                    bb = biasbp.tile([P, N], F32, name="bb", tag="bb")
                    nc.gpsimd.partition_broadcast(bb[:], b1[:])

                    wq_tiles = []
                    for t in range(KP):
                        wqt = wqp.tile([P, 2, N], FP8, name="wq", tag="wq")
                        for h in range(2):
                            src_t = stage_load(b, 2 * t + h, do_amax=False)
                            if nq % 3 == 0:
                                nc.scalar.activation(
                                    wqt[:, h, :], src_t[:],
                                    mybir.ActivationFunctionType.Copy, scale=sw_ap,
                                )
                            else:
                                nc.vector.tensor_scalar(
                                    wqt[:, h, :], src_t[:], sw_ap, None,
                                    op0=mybir.AluOpType.mult,
                                )
                            nq += 1
                        wq_tiles.append(wqt)

                    for mt in range(MT):
                        if mt % 2 == 0:
                            ost2 = ostp.tile([P, 2, N], BF16, name="ost", tag="ost")
                        ost = ost2[:, mt % 2, :]
                        psums = [
                            mmps.tile([P, NFREE], F32, name=f"mm{nt}", tag="mm")
                            for nt in range(NT)
                        ]
                        for t in range(KP):
                            lhsT = xqt[:, 2 * t:2 * t + 2,
                                       b * M + mt * P:b * M + (mt + 1) * P]
                            for nt in range(NT):
                                nc.tensor.matmul(
                                    psums[nt][:],
                                    lhsT,
                                    wq_tiles[t][:, :, nt * NFREE:(nt + 1) * NFREE],
                                    start=(t == 0),
                                    stop=(t == KP - 1),
                                    perf_mode=mybir.MatmulPerfMode.DoubleRow,
                                )
                        for nt in range(NT):
                            nc.vector.scalar_tensor_tensor(
                                ost[:, nt * NFREE:(nt + 1) * NFREE],
                                psums[nt][:],
                                c_ap,
                                bb[:, nt * NFREE:(nt + 1) * NFREE],
                                op0=mybir.AluOpType.mult,
                                op1=mybir.AluOpType.add,
                            )
                        if mt % 2 == 1:
                            nc.gpsimd.dma_start(
                                out[b, (mt - 1) * P:(mt + 1) * P, :].rearrange(
                                    "(p k) n -> k p n", p=2
                                ),
                                ost2[:],
                            )

    nc.compile()
    return nc


def _get_nc():
    if "fused" not in _cache:
        _cache["fused"] = _build_fused_nc()
    return _cache["fused"]


# test.py introspection: exec times (ns) of the last kernel() call.
last_run_info = {}


def kernel(input, weight, bias, _profile=False, _repeat=1, _trace_kwargs=None):
    input = np.ascontiguousarray(input, dtype=np.float32)
    weight = np.ascontiguousarray(weight, dtype=np.float32)
    bias = np.ascontiguousarray(bias, dtype=np.float32)
    assert input.shape == (B, M, K) and weight.shape == (B, K, N)
    assert bias.shape == (B, 1, N)

    consts = np.array([[FP8_HALF_MAX, 1.0]], dtype=np.float32)
    in_maps = [
        {
            "x": input[c * BL:(c + 1) * BL],
            "w": weight[c * BL:(c + 1) * BL],
            "bias": bias[c * BL:(c + 1) * BL],
            "consts": consts,
        }
        for c in range(NCORES)
    ]

    kw = dict(trace=_profile)
    if _trace_kwargs:
        kw.update(_trace_kwargs)

    nc = _get_nc()
    times = []
    res = None
    for _ in range(max(1, _repeat)):
        res = run_bass_kernel_spmd(nc, in_maps, core_ids=list(range(NCORES)), **kw)
        times.append(res.exec_time_ns)

    last_run_info.clear()
    last_run_info["amax_times"] = None
    last_run_info["mm_times"] = times
    last_run_info["amax_exec_ns"] = None
    last_run_info["mm_exec_ns"] = min(t for t in times if t) if any(times) else None
    last_run_info["mm_results"] = res

    out = np.concatenate(
        [np.asarray(res.results[c]["out"]).astype(np.float32) for c in range(NCORES)],
        axis=0,
    )
    return out
